# revision 16
# baseline (speedup 1.0000x reference)
"""Multi-head attention forward for TRN2, 8 NeuronCores, data-parallel over batch.

Reference (B=16, S=1024, D=768, H=12, HD=64), fp32:
    q = einsum('bsd,dhe->bshe', x, Wq) + bq        (same for k, v)
    z = einsum('bqhd,bkhd->bhqk', q/8, k)
    a = softmax(z, axis=-1)
    o = einsum('bhqk,bkhd->bqhd', a, v)
    y = einsum('bqhd,hde->bqe', o, Wo) + bo

Design notes (per core, 2 batches):
  - Host stages x pre-transposed (xT [D,S]) and all weights in bf16: the
    device does no input transposes and no dtype conversions.
  - Projections produce QT,KT [D,S] (head-transposed, bf16) and V with a
    ones column per head so the PV matmul accumulates the softmax
    denominator in PSUM column 64.
  - Scores transposed per head: zT[k,q] = KT_h.T @ QT_h (contraction 64);
    exp on ACT (scale=1/8 fused, no max-subtraction needed: |z| < ~3) into
    bf16 at tiles shaped [128, 2, 1024].
  - PV in [q,e] orientation: U[q,0:65] = sum_kt at(kt,qslice).T @ V(kt,h);
    col 64 is the denominator.  Eviction = reciprocal([128,1]) +
    tensor_scalar_mul: per-partition scalars, no partition broadcasts.
  - o -> oT via PE transposes (bf16 identity: 1 cycle/row); out-projection
    consumes oT with Wo slices as the moving operand.
  - Biases always folded at eviction (zero marginal cost).
  - The two batches are software-pipelined by explicit interleaved
    emission: batch 1 projections ride inside batch 0's (ACT-bound)
    attention span, batch 0's output projection rides inside batch 1's.
  - PSUM (8 banks): scores 2x[128,1024] + PV 2x[128,65] + transpose 1 +
    proj/outproj 1x[128,512].
"""

import numpy as np
from contextlib import ExitStack

import concourse.bacc as bacc
import concourse.bass as bass
import concourse.tile as tile
import concourse.mybir as mybir
from concourse.bass_utils import run_bass_kernel_spmd
from concourse.masks import make_identity

B, S, D, H, HD = 16, 1024, 768, 12, 64
NCORES = 8
BL = B // NCORES      # batches per core
P = 128
DC = D // P           # 6 contraction chunks
SQ = S // P           # 8 seq tiles of 128
F32 = mybir.dt.float32
BF16 = mybir.dt.bfloat16
F8 = mybir.dt.float8e4
DR = mybir.MatmulPerfMode.DoubleRow
EXP = mybir.ActivationFunctionType.Exp
SCALE = 1.0 / float(np.sqrt(HD))
ADD = mybir.AluOpType.add

_NC = {}


def _emit(tc, xT_d, w_d, b_d, y_d):
    nc = tc.nc

    with ExitStack() as ctx:
        consts = ctx.enter_context(tc.tile_pool(name="consts", bufs=1))
        wpool = ctx.enter_context(tc.tile_pool(name="wpool", bufs=1))
        big = ctx.enter_context(tc.tile_pool(name="big", bufs=1))
        atp = ctx.enter_context(tc.tile_pool(name="atp", bufs=6))
        opool = ctx.enter_context(tc.tile_pool(name="opool", bufs=2 * SQ))
        iop = ctx.enter_context(tc.tile_pool(name="iop", bufs=3))
        smal = ctx.enter_context(tc.tile_pool(name="smal", bufs=4))
        # PSUM pools: zz 2x2 + pv 2 + tp 1 + pr 1 = 8 banks
        zzp = ctx.enter_context(tc.tile_pool(name="zzp", bufs=2, space="PSUM"))
        pvp = ctx.enter_context(tc.tile_pool(name="pvp", bufs=2, space="PSUM"))
        tpp = ctx.enter_context(tc.tile_pool(name="tpp", bufs=1, space="PSUM"))
        prp = ctx.enter_context(tc.tile_pool(name="prp", bufs=1, space="PSUM"))

        # ---- constants ----
        ident = consts.tile([P, P], BF16)
        make_identity(nc, ident)
        bq_sb = consts.tile([P, DC], F32)
        nc.sync.dma_start(out=bq_sb, in_=b_d["bq"].rearrange("(c p) -> p c", p=P))
        bk_sb = consts.tile([P, DC], F32)
        nc.sync.dma_start(out=bk_sb, in_=b_d["bk"].rearrange("(c p) -> p c", p=P))
        brow = consts.tile([2, D], F32)
        nc.sync.dma_start(out=brow[0:1, :], in_=b_d["bv"].unsqueeze(0))
        nc.sync.dma_start(out=brow[1:2, :], in_=b_d["bo"].unsqueeze(0))
        bvb = consts.tile([P, D], BF16)
        bob = consts.tile([P, D], BF16)
        for i, dst in enumerate((bvb, bob)):
            srow = brow[i:i + 1, :]
            srcap = bass.AP(tensor=srow.tensor, offset=srow.offset,
                            ap=[list(srow.ap[0]), [0, P], list(srow.ap[1])])
            nc.gpsimd.dma_start(out=dst, in_=srcap)
        # warm the ACT exp table at t=0 (overlaps the initial DMAs)
        expwarm = consts.tile([1, 1], F32)
        nc.scalar.activation(expwarm, bq_sb[0:1, 0:1], EXP)

        # ---- input DMAs, ordered so head 0 of batch 0 unblocks earliest ----
        xT, w_sb = [], {}

        def dma_in(tile_ap, src, c):
            nc.sync.dma_start(out=tile_ap[:, c:c + 2, :], in_=src[:, c:c + 2, :])

        for b in range(BL):
            xT.append(big.tile([P, DC, S], BF16, tag="xT", name=f"xT_{b}",
                               bufs=BL))
        for name in ("wq", "wk", "wv", "wo"):
            w_sb[name] = wpool.tile([P, DC, D], BF16, name=f"w_{name}")
        xsrc = [xT_d[b].rearrange("(c p) s -> p c s", p=P) for b in range(BL)]
        wsrc = {n: w_d[n].rearrange("(c p) m -> p c m", p=P)
                for n in ("wq", "wk", "wv", "wo")}
        # interleave chunk DMAs so the first projection's accumulation can
        # chase the arrivals instead of waiting for whole tensors
        for c in range(0, DC, 2):
            dma_in(xT[0], xsrc[0], c)
            dma_in(w_sb["wq"], wsrc["wq"], c)
            dma_in(w_sb["wk"], wsrc["wk"], c)
        for c in range(0, DC, 2):
            dma_in(w_sb["wv"], wsrc["wv"], c)
        for c in range(0, DC, 2):
            dma_in(xT[1], xsrc[1], c)
            dma_in(w_sb["wo"], wsrc["wo"], c)

        # ---- per-batch tensors ----
        # QT8/KT8: fp8 DoubleRow layout for scores.  Head h lives at
        # partitions 32*(h%4)..+32, group hg=h//4; dim2 is the contraction
        # pair (head-dim d = 32*pair + row).
        QT8 = [big.tile([P, 3, 2, S], F8, tag="QT", name=f"QT_{b}", bufs=BL)
               for b in range(BL)]
        KT8 = [big.tile([P, 3, 2, S], F8, tag="KT", name=f"KT_{b}", bufs=BL)
               for b in range(BL)]
        V = [big.tile([P, SQ, H, 65], BF16, tag="V", name=f"V_{b}", bufs=BL)
             for b in range(BL)]
        for b in range(BL):
            nc.gpsimd.memset(V[b][:, :, :, 64], 1.0)
        o_t = [[opool.tile([P, D], BF16, tag="o", name=f"o_{b}_{qt}")
                for qt in range(SQ)] for b in range(BL)]
        oT = [big.tile([P, DC, S], BF16, tag="oT", name=f"oT_{b}", bufs=1)
              for b in range(BL)]
        ats = {}

        # ---- emission units ----
        def u_qk(b, m):
            # m-tile holds heads 2m (psum rows 0:64) and 2m+1 (rows 64:128)
            for wname, bcol, out in (("wq", bq_sb, QT8[b]), ("wk", bk_sb, KT8[b])):
                stg = iop.tile([P, S], F8, tag=f"st{wname}",
                               name=f"st{wname}_{b}_{m}", bufs=2)
                for sh in range(2):
                    ps = prp.tile([P, 512], F32, tag="pr",
                                  name=f"p{wname}_{b}_{m}_{sh}")
                    w = w_sb[wname]
                    for c in range(DC):
                        nc.tensor.matmul(
                            ps, w[:, c, m * P:(m + 1) * P],
                            xT[b][:, c, sh * 512:(sh + 1) * 512],
                            start=(c == 0), stop=(c == DC - 1))
                    nc.vector.tensor_scalar_add(
                        stg[:, sh * 512:(sh + 1) * 512], ps, bcol[:, m:m + 1])
                # rearrange into DoubleRow layout (partition remap DMAs,
                # split across the gpsimd SWDGE and ACT HWDGE queues)
                eng = nc.gpsimd if wname == "wq" else nc.scalar
                for r in range(2):
                    h = 2 * m + r
                    hg, rb = h // 4, 32 * (h % 4)
                    for j in range(2):
                        eng.dma_start(
                            out=out[rb:rb + 32, hg, j, :],
                            in_=stg[64 * r + 32 * j:64 * r + 32 * (j + 1), :])

        def u_v(b, dh, st):
            n = 512 if dh == 0 else 256
            ps = prp.tile([P, 512], F32, tag="pr", name=f"pv_{b}_{st}_{dh}")
            for c in range(DC):
                nc.tensor.matmul(
                    ps[:, 0:n], xT[b][:, c, st * P:(st + 1) * P],
                    w_sb["wv"][:, c, dh * 512:dh * 512 + n],
                    start=(c == 0), stop=(c == DC - 1))
            h0, nh = dh * 8, n // HD
            nc.vector.tensor_tensor(
                out=V[b][:, st, h0:h0 + nh, 0:HD],
                in0=ps[:, 0:n].rearrange("p (h e) -> p h e", h=nh),
                in1=bvb[:, dh * 512:dh * 512 + n].rearrange(
                    "p (h e) -> p h e", h=nh),
                op=ADD)

        def u_scores(b, h):
            hg, rb = h // 4, 32 * (h % 4)
            rsl = slice(rb, rb + 32)
            at = [atp.tile([P, 2, S], BF16, tag="at", name=f"at_{b}_{h}_{p}")
                  for p in range(4)]
            ats[(b, h)] = at
            for ktp in range(4):
                for j in range(2):
                    kt = 2 * ktp + j
                    zz = zzp.tile([P, S], F32, tag="zz", name=f"zz_{b}_{h}_{kt}")
                    for hf in range(2):
                        nc.tensor.matmul(
                            zz[:, hf * 512:(hf + 1) * 512],
                            KT8[b][rsl, hg, :, kt * P:(kt + 1) * P],
                            QT8[b][rsl, hg, :, hf * 512:(hf + 1) * 512],
                            start=True, stop=True, perf_mode=DR,
                            tile_position=(rb, 0))
                    nc.scalar.activation(at[ktp][:, j, :], zz, EXP, scale=SCALE)

        def u_pv(b, h):
            at = ats.pop((b, h))
            for qt in range(SQ):
                pv = pvp.tile([P, 65], F32, tag="pv", name=f"pv_{b}_{h}_{qt}")
                for ktp in range(4):
                    for j in range(2):
                        nc.tensor.matmul(
                            pv, at[ktp][:, j, qt * P:(qt + 1) * P],
                            V[b][:, 2 * ktp + j, h, :],
                            start=(ktp == 0 and j == 0),
                            stop=(ktp == 3 and j == 1))
                rd = smal.tile([P, 1], F32, tag="rd", name=f"rd_{b}_{h}_{qt}")
                nc.vector.reciprocal(rd, pv[:, HD:HD + 1])
                nc.vector.tensor_scalar_mul(
                    o_t[b][qt][:, h * HD:(h + 1) * HD], pv[:, 0:HD], rd)

        def u_d(b, qt):
            tp = tpp.tile([P, D], BF16, tag="tp", name=f"tp_{b}_{qt}")
            for c in range(DC):
                nc.tensor.transpose(
                    tp[:, c * P:(c + 1) * P],
                    o_t[b][qt][:, c * P:(c + 1) * P], ident)
            nc.vector.tensor_copy(
                oT[b][:, :, qt * P:(qt + 1) * P],
                tp.rearrange("p (c q) -> p c q", c=DC))
            yst = iop.tile([P, D], F32, tag="yst", name=f"y_{b}_{qt}")
            y_b = y_d[b].rearrange("(t p) d -> p t d", p=P)
            # last tile: store halves as they finish (shorter kernel tail)
            split = (b == BL - 1 and qt >= SQ - 2)
            for dh in range(2):
                n = 512 if dh == 0 else 256
                ps = prp.tile([P, 512], F32, tag="pr", name=f"py_{b}_{qt}_{dh}")
                for c in range(DC):
                    nc.tensor.matmul(
                        ps[:, 0:n], oT[b][:, c, qt * P:(qt + 1) * P],
                        w_sb["wo"][:, c, dh * 512:dh * 512 + n],
                        start=(c == 0), stop=(c == DC - 1))
                nc.vector.tensor_tensor(
                    out=yst[:, dh * 512:dh * 512 + n], in0=ps[:, 0:n],
                    in1=bob[:, dh * 512:dh * 512 + n], op=ADD)
                if split:
                    nc.sync.dma_start(out=y_b[:, qt, dh * 512:dh * 512 + n],
                                      in_=yst[:, dh * 512:dh * 512 + n])
            if not split:
                nc.sync.dma_start(out=y_b[:, qt, :], in_=yst)

        # ---- interleaved schedule ----
        # C(b,h) = scores+exp then PV for head h; projections of the other
        # batch and the finished batch's output projection ride between
        # heads so the PE never starves while ACT (exp) paces the kernel.
        sched = []
        sched += [("qk", 0, 0), ("qk", 0, 1)]
        sched += [("v", 0, 0, st) for st in range(SQ)]
        sched += [("C", 0, 0), ("qk", 0, 2), ("C", 0, 1)]
        sched += [("v", 0, 1, st) for st in range(4)]
        sched += [("C", 0, 2), ("qk", 0, 3), ("C", 0, 3)]
        sched += [("v", 0, 1, st) for st in range(4, SQ)]
        sched += [("C", 0, 4), ("qk", 0, 4), ("C", 0, 5), ("qk", 0, 5),
                  ("C", 0, 6), ("qk", 1, 0),
                  ("C", 0, 7), ("qk", 1, 1),
                  ("C", 0, 8)] + [("v", 1, 0, st) for st in range(SQ)]
        sched += [("C", 0, 9), ("qk", 1, 2),
                  ("C", 0, 10)] + [("v", 1, 1, st) for st in range(SQ)]
        sched += [("C", 0, 11), ("qk", 1, 3)]
        sched += [("D", 0, 0), ("C", 1, 0), ("D", 0, 1), ("C", 1, 1),
                  ("qk", 1, 4),
                  ("D", 0, 2), ("C", 1, 2), ("D", 0, 3), ("C", 1, 3),
                  ("qk", 1, 5),
                  ("D", 0, 4), ("C", 1, 4), ("D", 0, 5), ("C", 1, 5),
                  ("D", 0, 6), ("C", 1, 6), ("D", 0, 7), ("C", 1, 7),
                  ("C", 1, 8), ("C", 1, 9), ("C", 1, 10), ("C", 1, 11)]
        sched += [("D", 1, qt) for qt in range(SQ)]

        for unit in sched:
            kind = unit[0]
            if kind == "qk":
                u_qk(unit[1], unit[2])
            elif kind == "v":
                u_v(unit[1], unit[2], unit[3])
            elif kind == "C":
                u_scores(unit[1], unit[2])
                u_pv(unit[1], unit[2])
            elif kind == "D":
                u_d(unit[1], unit[2])


def _build():
    nc = bacc.Bacc("TRN2", target_bir_lowering=False, debug=False,
                   num_devices=NCORES)
    xT_d = nc.dram_tensor("xT", [BL, D, S], BF16, kind="ExternalInput").ap()
    w_d = {n: nc.dram_tensor(n, [D, D], BF16, kind="ExternalInput").ap()
           for n in ("wq", "wk", "wv", "wo")}
    b_d = {n: nc.dram_tensor(n, [D], F32, kind="ExternalInput").ap()
           for n in ("bq", "bk", "bv", "bo")}
    y_d = nc.dram_tensor("y", [BL, S, D], F32, kind="ExternalOutput").ap()
    with tile.TileContext(nc) as tc:
        _emit(tc, xT_d, w_d, b_d, y_d)
    nc.compile()
    return nc


def _in_maps(x, Wq, bq, Wk, bk, Wv, bv, Wo, bo):
    import ml_dtypes
    bf = ml_dtypes.bfloat16

    def _w(a):
        return np.ascontiguousarray(
            np.asarray(a, dtype=np.float32).reshape(D, D).astype(bf))

    def _b(a):
        return np.ascontiguousarray(np.asarray(a, dtype=np.float32).reshape(D))

    w = {"wq": _w(Wq), "wk": _w(Wk), "wv": _w(Wv), "wo": _w(Wo),
         "bq": _b(bq), "bk": _b(bk), "bv": _b(bv), "bo": _b(bo)}
    xT = np.asarray(x, dtype=np.float32).transpose(0, 2, 1).astype(bf)
    return [dict(w, xT=np.ascontiguousarray(xT[i * BL:(i + 1) * BL]))
            for i in range(NCORES)]


def get_nc(with_bias=True):
    if 0 not in _NC:
        _NC[0] = _build()
    return _NC[0]


def run(inputs, trace=False):
    nc = get_nc()
    maps = _in_maps(**inputs)
    res = run_bass_kernel_spmd(nc, maps, list(range(NCORES)), trace=trace)
    y = np.concatenate([res.results[i]["y"] for i in range(NCORES)], axis=0)
    return y, res


def kernel(x, Wq, bq, Wk, bk, Wv, bv, Wo, bo):
    y, _ = run(dict(x=x, Wq=Wq, bq=bq, Wk=Wk, bk=bk, Wv=Wv, bv=bv,
                    Wo=Wo, bo=bo))
    return y


# revision 17
# speedup vs baseline: 1.0434x; 1.0434x over previous
"""Multi-head attention forward for TRN2, 8 NeuronCores, data-parallel over batch.

Reference (B=16, S=1024, D=768, H=12, HD=64), fp32:
    q = einsum('bsd,dhe->bshe', x, Wq) + bq        (same for k, v)
    z = einsum('bqhd,bkhd->bhqk', q/8, k)
    a = softmax(z, axis=-1)
    o = einsum('bhqk,bkhd->bqhd', a, v)
    y = einsum('bqhd,hde->bqe', o, Wo) + bo

Design notes (per core, 2 batches):
  - Host stages x pre-transposed (xT [D,S]) and all weights in bf16: the
    device does no input transposes and no dtype conversions.
  - Projections produce QT,KT [D,S] (head-transposed, bf16) and V with a
    ones column per head so the PV matmul accumulates the softmax
    denominator in PSUM column 64.
  - Scores transposed per head: zT[k,q] = KT_h.T @ QT_h (contraction 64);
    exp on ACT (scale=1/8 fused, no max-subtraction needed: |z| < ~3) into
    bf16 at tiles shaped [128, 2, 1024].
  - PV in [q,e] orientation: U[q,0:65] = sum_kt at(kt,qslice).T @ V(kt,h);
    col 64 is the denominator.  Eviction = reciprocal([128,1]) +
    tensor_scalar_mul: per-partition scalars, no partition broadcasts.
  - o -> oT via PE transposes (bf16 identity: 1 cycle/row); out-projection
    consumes oT with Wo slices as the moving operand.
  - Biases always folded at eviction (zero marginal cost).
  - The two batches are software-pipelined by explicit interleaved
    emission: batch 1 projections ride inside batch 0's (ACT-bound)
    attention span, batch 0's output projection rides inside batch 1's.
  - PSUM (8 banks): scores 2x[128,1024] + PV 2x[128,65] + transpose 1 +
    proj/outproj 1x[128,512].
"""

import numpy as np
from contextlib import ExitStack

import concourse.bacc as bacc
import concourse.bass as bass
import concourse.tile as tile
import concourse.mybir as mybir
from concourse.bass_utils import run_bass_kernel_spmd
from concourse.masks import make_identity

B, S, D, H, HD = 16, 1024, 768, 12, 64
NCORES = 8
BL = B // NCORES      # batches per core
P = 128
DC = D // P           # 6 contraction chunks
SQ = S // P           # 8 seq tiles of 128
F32 = mybir.dt.float32
BF16 = mybir.dt.bfloat16
F8 = mybir.dt.float8e4
DR = mybir.MatmulPerfMode.DoubleRow
EXP = mybir.ActivationFunctionType.Exp
SCALE = 1.0 / float(np.sqrt(HD))
ADD = mybir.AluOpType.add

_NC = {}


def _emit(tc, xT_d, w_d, b_d, y_d):
    nc = tc.nc

    with ExitStack() as ctx:
        consts = ctx.enter_context(tc.tile_pool(name="consts", bufs=1))
        wpool = ctx.enter_context(tc.tile_pool(name="wpool", bufs=1))
        big = ctx.enter_context(tc.tile_pool(name="big", bufs=1))
        atp = ctx.enter_context(tc.tile_pool(name="atp", bufs=6))
        opool = ctx.enter_context(tc.tile_pool(name="opool", bufs=2 * SQ))
        iop = ctx.enter_context(tc.tile_pool(name="iop", bufs=3))
        smal = ctx.enter_context(tc.tile_pool(name="smal", bufs=4))
        # PSUM pools: zz 2x2 + pv 2 + tp 1 + pr 1 = 8 banks
        zzp = ctx.enter_context(tc.tile_pool(name="zzp", bufs=2, space="PSUM"))
        pvp = ctx.enter_context(tc.tile_pool(name="pvp", bufs=2, space="PSUM"))
        tpp = ctx.enter_context(tc.tile_pool(name="tpp", bufs=1, space="PSUM"))
        prp = ctx.enter_context(tc.tile_pool(name="prp", bufs=1, space="PSUM"))

        # ---- constants ----
        ident = consts.tile([P, P], BF16)
        make_identity(nc, ident)
        bq_sb = consts.tile([P, DC], F32)
        nc.sync.dma_start(out=bq_sb, in_=b_d["bq"].rearrange("(c p) -> p c", p=P))
        bk_sb = consts.tile([P, DC], F32)
        nc.sync.dma_start(out=bk_sb, in_=b_d["bk"].rearrange("(c p) -> p c", p=P))
        brow = consts.tile([2, D], F32)
        nc.sync.dma_start(out=brow[0:1, :], in_=b_d["bv"].unsqueeze(0))
        nc.sync.dma_start(out=brow[1:2, :], in_=b_d["bo"].unsqueeze(0))
        bvb = consts.tile([P, D], BF16)
        bob = consts.tile([P, D], BF16)
        for i, dst in enumerate((bvb, bob)):
            srow = brow[i:i + 1, :]
            srcap = bass.AP(tensor=srow.tensor, offset=srow.offset,
                            ap=[list(srow.ap[0]), [0, P], list(srow.ap[1])])
            nc.gpsimd.dma_start(out=dst, in_=srcap)
        # warm the ACT exp table at t=0 (overlaps the initial DMAs)
        expwarm = consts.tile([1, 1], F32)
        nc.scalar.activation(expwarm, bq_sb[0:1, 0:1], EXP)

        # ---- input DMAs, ordered so head 0 of batch 0 unblocks earliest ----
        xT, w_sb = [], {}

        def dma_in(tile_ap, src, c):
            nc.sync.dma_start(out=tile_ap[:, c:c + 2, :], in_=src[:, c:c + 2, :])

        for b in range(BL):
            xT.append(big.tile([P, DC, S], BF16, tag="xT", name=f"xT_{b}",
                               bufs=BL))
        for name in ("wq", "wk", "wv", "wo"):
            w_sb[name] = wpool.tile([P, DC, D], BF16, name=f"w_{name}")
        xsrc = [xT_d[b].rearrange("(c p) s -> p c s", p=P) for b in range(BL)]
        wsrc = {n: w_d[n].rearrange("(c p) m -> p c m", p=P)
                for n in ("wq", "wk", "wv", "wo")}
        # interleave chunk DMAs so the first projection's accumulation can
        # chase the arrivals instead of waiting for whole tensors
        for c in range(0, DC, 2):
            dma_in(xT[0], xsrc[0], c)
            dma_in(w_sb["wq"], wsrc["wq"], c)
            dma_in(w_sb["wk"], wsrc["wk"], c)
        for c in range(0, DC, 2):
            dma_in(w_sb["wv"], wsrc["wv"], c)
        for c in range(0, DC, 2):
            dma_in(xT[1], xsrc[1], c)
            dma_in(w_sb["wo"], wsrc["wo"], c)

        # ---- per-batch tensors ----
        # QT8/KT8: fp8 DoubleRow layout for scores.  Head h lives at
        # partitions 32*(h%4)..+32, group hg=h//4; dim2 is the contraction
        # pair (head-dim d = 32*pair + row).
        QT8 = [big.tile([P, 3, 2, S], F8, tag="QT", name=f"QT_{b}", bufs=BL)
               for b in range(BL)]
        KT8 = [big.tile([P, 3, 2, S], F8, tag="KT", name=f"KT_{b}", bufs=BL)
               for b in range(BL)]
        V = [big.tile([P, SQ, H, 65], BF16, tag="V", name=f"V_{b}", bufs=BL)
             for b in range(BL)]
        for b in range(BL):
            nc.gpsimd.memset(V[b][:, :, :, 64], 1.0)
        o_t = [[opool.tile([P, D], BF16, tag="o", name=f"o_{b}_{qt}")
                for qt in range(SQ)] for b in range(BL)]
        oT = [big.tile([P, DC, S], BF16, tag="oT", name=f"oT_{b}", bufs=1)
              for b in range(BL)]
        ats = {}

        # ---- emission units ----
        def u_qk(b, m):
            # m-tile holds heads 2m (psum rows 0:64) and 2m+1 (rows 64:128)
            for wname, bcol, out in (("wq", bq_sb, QT8[b]), ("wk", bk_sb, KT8[b])):
                stg = iop.tile([P, S], F8, tag=f"st{wname}",
                               name=f"st{wname}_{b}_{m}", bufs=2)
                for sh in range(2):
                    ps = prp.tile([P, 512], F32, tag="pr",
                                  name=f"p{wname}_{b}_{m}_{sh}")
                    w = w_sb[wname]
                    for c in range(DC):
                        nc.tensor.matmul(
                            ps, w[:, c, m * P:(m + 1) * P],
                            xT[b][:, c, sh * 512:(sh + 1) * 512],
                            start=(c == 0), stop=(c == DC - 1))
                    nc.vector.tensor_scalar_add(
                        stg[:, sh * 512:(sh + 1) * 512], ps, bcol[:, m:m + 1])
                # rearrange into DoubleRow layout (partition remap DMAs,
                # split across the gpsimd SWDGE and ACT HWDGE queues)
                eng = nc.gpsimd if wname == "wq" else nc.sync
                for r in range(2):
                    h = 2 * m + r
                    hg, rb = h // 4, 32 * (h % 4)
                    for j in range(2):
                        eng.dma_start(
                            out=out[rb:rb + 32, hg, j, :],
                            in_=stg[64 * r + 32 * j:64 * r + 32 * (j + 1), :])

        def u_v(b, dh, st):
            n = 512 if dh == 0 else 256
            ps = prp.tile([P, 512], F32, tag="pr", name=f"pv_{b}_{st}_{dh}")
            for c in range(DC):
                nc.tensor.matmul(
                    ps[:, 0:n], xT[b][:, c, st * P:(st + 1) * P],
                    w_sb["wv"][:, c, dh * 512:dh * 512 + n],
                    start=(c == 0), stop=(c == DC - 1))
            h0, nh = dh * 8, n // HD
            nc.vector.tensor_tensor(
                out=V[b][:, st, h0:h0 + nh, 0:HD],
                in0=ps[:, 0:n].rearrange("p (h e) -> p h e", h=nh),
                in1=bvb[:, dh * 512:dh * 512 + n].rearrange(
                    "p (h e) -> p h e", h=nh),
                op=ADD)

        def u_scores(b, h):
            hg, rb = h // 4, 32 * (h % 4)
            rsl = slice(rb, rb + 32)
            at = [atp.tile([P, 2, S], BF16, tag="at", name=f"at_{b}_{h}_{p}")
                  for p in range(4)]
            ats[(b, h)] = at
            for ktp in range(4):
                for j in range(2):
                    kt = 2 * ktp + j
                    zz = zzp.tile([P, S], F32, tag="zz", name=f"zz_{b}_{h}_{kt}")
                    for hf in range(2):
                        nc.tensor.matmul(
                            zz[:, hf * 512:(hf + 1) * 512],
                            KT8[b][rsl, hg, :, kt * P:(kt + 1) * P],
                            QT8[b][rsl, hg, :, hf * 512:(hf + 1) * 512],
                            start=True, stop=True, perf_mode=DR,
                            tile_position=(rb, 0))
                    nc.scalar.activation(at[ktp][:, j, :], zz, EXP, scale=SCALE)

        def u_pv(b, h):
            at = ats.pop((b, h))
            for qt in range(SQ):
                pv = pvp.tile([P, 65], F32, tag="pv", name=f"pv_{b}_{h}_{qt}")
                for ktp in range(4):
                    for j in range(2):
                        nc.tensor.matmul(
                            pv, at[ktp][:, j, qt * P:(qt + 1) * P],
                            V[b][:, 2 * ktp + j, h, :],
                            start=(ktp == 0 and j == 0),
                            stop=(ktp == 3 and j == 1))
                rd = smal.tile([P, 1], F32, tag="rd", name=f"rd_{b}_{h}_{qt}")
                nc.vector.reciprocal(rd, pv[:, HD:HD + 1])
                nc.vector.tensor_scalar_mul(
                    o_t[b][qt][:, h * HD:(h + 1) * HD], pv[:, 0:HD], rd)

        def u_d(b, qt):
            tp = tpp.tile([P, D], BF16, tag="tp", name=f"tp_{b}_{qt}")
            for c in range(DC):
                nc.tensor.transpose(
                    tp[:, c * P:(c + 1) * P],
                    o_t[b][qt][:, c * P:(c + 1) * P], ident)
            nc.vector.tensor_copy(
                oT[b][:, :, qt * P:(qt + 1) * P],
                tp.rearrange("p (c q) -> p c q", c=DC))
            yst = iop.tile([P, D], F32, tag="yst", name=f"y_{b}_{qt}")
            y_b = y_d[b].rearrange("(t p) d -> p t d", p=P)
            # last tile: store halves as they finish (shorter kernel tail)
            split = (b == BL - 1 and qt >= SQ - 2)
            for dh in range(2):
                n = 512 if dh == 0 else 256
                ps = prp.tile([P, 512], F32, tag="pr", name=f"py_{b}_{qt}_{dh}")
                for c in range(DC):
                    nc.tensor.matmul(
                        ps[:, 0:n], oT[b][:, c, qt * P:(qt + 1) * P],
                        w_sb["wo"][:, c, dh * 512:dh * 512 + n],
                        start=(c == 0), stop=(c == DC - 1))
                nc.vector.tensor_tensor(
                    out=yst[:, dh * 512:dh * 512 + n], in0=ps[:, 0:n],
                    in1=bob[:, dh * 512:dh * 512 + n], op=ADD)
                if split:
                    nc.sync.dma_start(out=y_b[:, qt, dh * 512:dh * 512 + n],
                                      in_=yst[:, dh * 512:dh * 512 + n])
            if not split:
                nc.sync.dma_start(out=y_b[:, qt, :], in_=yst)

        # ---- interleaved schedule ----
        # C(b,h) = scores+exp then PV for head h; projections of the other
        # batch and the finished batch's output projection ride between
        # heads so the PE never starves while ACT (exp) paces the kernel.
        sched = []
        sched += [("qk", 0, 0), ("qk", 0, 1)]
        sched += [("v", 0, 0, st) for st in range(SQ)]
        sched += [("C", 0, 0), ("qk", 0, 2), ("C", 0, 1)]
        sched += [("v", 0, 1, st) for st in range(4)]
        sched += [("C", 0, 2), ("qk", 0, 3), ("C", 0, 3)]
        sched += [("v", 0, 1, st) for st in range(4, SQ)]
        sched += [("C", 0, 4), ("qk", 0, 4), ("C", 0, 5), ("qk", 0, 5),
                  ("C", 0, 6), ("qk", 1, 0),
                  ("C", 0, 7), ("qk", 1, 1),
                  ("C", 0, 8)] + [("v", 1, 0, st) for st in range(SQ)]
        sched += [("C", 0, 9), ("qk", 1, 2),
                  ("C", 0, 10)] + [("v", 1, 1, st) for st in range(SQ)]
        sched += [("C", 0, 11), ("qk", 1, 3)]
        sched += [("D", 0, 0), ("C", 1, 0), ("D", 0, 1), ("C", 1, 1),
                  ("qk", 1, 4),
                  ("D", 0, 2), ("C", 1, 2), ("D", 0, 3), ("C", 1, 3),
                  ("qk", 1, 5),
                  ("D", 0, 4), ("C", 1, 4), ("D", 0, 5), ("C", 1, 5),
                  ("D", 0, 6), ("C", 1, 6), ("D", 0, 7), ("C", 1, 7),
                  ("C", 1, 8), ("C", 1, 9), ("C", 1, 10), ("C", 1, 11)]
        sched += [("D", 1, qt) for qt in range(SQ)]

        for unit in sched:
            kind = unit[0]
            if kind == "qk":
                u_qk(unit[1], unit[2])
            elif kind == "v":
                u_v(unit[1], unit[2], unit[3])
            elif kind == "C":
                u_scores(unit[1], unit[2])
                u_pv(unit[1], unit[2])
            elif kind == "D":
                u_d(unit[1], unit[2])


def _build():
    nc = bacc.Bacc("TRN2", target_bir_lowering=False, debug=False,
                   num_devices=NCORES)
    xT_d = nc.dram_tensor("xT", [BL, D, S], BF16, kind="ExternalInput").ap()
    w_d = {n: nc.dram_tensor(n, [D, D], BF16, kind="ExternalInput").ap()
           for n in ("wq", "wk", "wv", "wo")}
    b_d = {n: nc.dram_tensor(n, [D], F32, kind="ExternalInput").ap()
           for n in ("bq", "bk", "bv", "bo")}
    y_d = nc.dram_tensor("y", [BL, S, D], F32, kind="ExternalOutput").ap()
    with tile.TileContext(nc) as tc:
        _emit(tc, xT_d, w_d, b_d, y_d)
    nc.compile()
    return nc


def _in_maps(x, Wq, bq, Wk, bk, Wv, bv, Wo, bo):
    import ml_dtypes
    bf = ml_dtypes.bfloat16

    def _w(a):
        return np.ascontiguousarray(
            np.asarray(a, dtype=np.float32).reshape(D, D).astype(bf))

    def _b(a):
        return np.ascontiguousarray(np.asarray(a, dtype=np.float32).reshape(D))

    w = {"wq": _w(Wq), "wk": _w(Wk), "wv": _w(Wv), "wo": _w(Wo),
         "bq": _b(bq), "bk": _b(bk), "bv": _b(bv), "bo": _b(bo)}
    xT = np.asarray(x, dtype=np.float32).transpose(0, 2, 1).astype(bf)
    return [dict(w, xT=np.ascontiguousarray(xT[i * BL:(i + 1) * BL]))
            for i in range(NCORES)]


def get_nc(with_bias=True):
    if 0 not in _NC:
        _NC[0] = _build()
    return _NC[0]


def run(inputs, trace=False):
    nc = get_nc()
    maps = _in_maps(**inputs)
    res = run_bass_kernel_spmd(nc, maps, list(range(NCORES)), trace=trace)
    y = np.concatenate([res.results[i]["y"] for i in range(NCORES)], axis=0)
    return y, res


def kernel(x, Wq, bq, Wk, bk, Wv, bv, Wo, bo):
    y, _ = run(dict(x=x, Wq=Wq, bq=bq, Wk=Wk, bk=bk, Wv=Wv, bv=bv,
                    Wo=Wo, bo=bo))
    return y
